# revision 1
# baseline (speedup 1.0000x reference)
"""EqPBCNN (perturbation-based nonlinearity compensation NN) Trainium2 Bass kernel.

Data-parallel over 8 NeuronCores: batch 65536 -> 8192 per core.

Math (per sample, per polarization p):
  triplet features  F[h,p] = SYM[h] * (A[h,0]+A[h,1]) * x[m_h,p],
                    A[h,p] = x[n_h,p] * conj(x[m_h+n_h,p])
  h1 = CLrelu(F @ W1^T); h2 = CLrelu(h1 @ W2^T); E = h2 @ W3^T
  out = x[center,p] + E * 10^(task0/10)/2

Kernel pipeline (taps-on-partitions, batch on free dim, chunks of 512):
  gather matmuls (PE)  -> pair stacks A,C (350 rows = (h, pol))
  G products (DVE)     -> G = A * conj(C)
  R matmuls (PE)       -> R[o,m,p] = sum_n W1'[p,o,(m,n)] * (G[h,0]+G[h,1])   (100 rows)
  T products (DVE)     -> T = xrep * R   (complex)
  final matmul (PE)    -> h1[p,o] = sum_m T    (8 rows)
  ACT lrelu / W2 / lrelu / W3 -> E (4 rows); exp for P; residual add; store.
"""
import numpy as np

# ---------------- problem constants (hardcoded; must match reference) -------
BATCH = 65536
MT, LH = 41, 20          # filter taps, half window
NM = 2                   # modes / polarizations
H1, H2 = 2, 10
SLOPE = 0.01
NCORES = 8
BCORE = BATCH // NCORES  # 8192
NB = 512                 # samples per chunk
NCHUNK = BCORE // NB     # 16
ROWS = MT * NM           # 82 = tap*2 + mode

_idx = [(m, n) for m in range(-LH, LH + 1) for n in range(-LH, LH + 1)
        if abs(m * n) <= LH and abs(m + n) <= LH and n >= m]
H = len(_idx)            # 175
M_ARR = np.array([t[0] for t in _idx], np.int32)
N_ARR = np.array([t[1] for t in _idx], np.int32)
A_TAP = N_ARR + LH           # source tap for En
C_TAP = M_ARR + N_ARR + LH   # source tap for Emn (conjugated side)
SYM = np.where(M_ARR != N_ARR, 2.0, 1.0).astype(np.float32)
M_VALS = sorted(set(M_ARR.tolist()))     # 25 distinct m values
NMV = len(M_VALS)
M_POS = {m: i for i, m in enumerate(M_VALS)}
NO = H1 * NMV * NM       # 100 rows of R/T space: (o, mi, p)
NSTACK = 2 * H           # 350 rows: (h, pol)
KSPLITS = [(0, 128), (128, 128), (256, NSTACK - 256)]   # psplits of the stacks


def _orow(o, mi, p):
    return (o * NMV + mi) * NM + p


def _hrow(p, o, comp):
    return (p * H1 + o) * 2 + comp


def _h2row(p, q, comp):
    return (p * H2 + q) * 2 + comp


def build_static():
    """Weight-independent constant matrices."""
    # gather selections: stack row r = 2h+p reads XT row 2*tap+p
    SEL = np.zeros((ROWS, 2 * NSTACK), np.float32)   # [82, 700]: cols 0:350 A, 350:700 C
    for h in range(H):
        for p in range(NM):
            r = 2 * h + p
            SEL[2 * A_TAP[h] + p, r] = 1.0
            SEL[2 * C_TAP[h] + p, NSTACK + r] = 1.0
    # xrep: col (o,mi,p) reads tap m
    XREPW = np.zeros((ROWS, NO), np.float32)
    for o in range(H1):
        for mi, mv in enumerate(M_VALS):
            for p in range(NM):
                XREPW[2 * (mv + LH) + p, _orow(o, mi, p)] = 1.0
    # final contraction [100, 16]: cols 0:8 from Tre, 8:16 from Tim
    FINW = np.zeros((NO, 16), np.float32)
    for o in range(H1):
        for mi in range(NMV):
            for p in range(NM):
                FINW[_orow(o, mi, p), _hrow(p, o, 0)] = 1.0
                FINW[_orow(o, mi, p), 8 + _hrow(p, o, 1)] = 1.0
    return {"SEL": SEL, "XREPW": XREPW, "FINW": FINW}


def fold_weights(W1r, W1i, W2r, W2i, W3r, W3i):
    """Runtime weight folding into matmul lhsT constants."""
    Wr = W1r * SYM[None, None, :]   # [p, o, h]
    Wi = W1i * SYM[None, None, :]
    # R lhsT: [350, 400] cols: 0:100 Gr->Rre, 100:200 Gi->Rre, 200:300 Gr->Rim, 300:400 Gi->Rim
    RW = np.zeros((NSTACK, 4 * NO), np.float32)
    for h in range(H):
        mi = M_POS[M_ARR[h]]
        for p in range(NM):          # output pol (weights are per-pol)
            for q in range(NM):      # source stack row pol (pol-swap fold)
                r = 2 * h + q
                for o in range(H1):
                    c = _orow(o, mi, p)
                    RW[r, 0 * NO + c] += Wr[p, o, h]
                    RW[r, 1 * NO + c] -= Wi[p, o, h]
                    RW[r, 2 * NO + c] += Wi[p, o, h]
                    RW[r, 3 * NO + c] += Wr[p, o, h]
    RWP = np.zeros((3, 128, 4 * NO), np.float32)
    for k, (r0, rk) in enumerate(KSPLITS):
        RWP[k, :rk, :] = RW[r0:r0 + rk, :]
    # W2 lhsT [8, 40]
    W2L = np.zeros((8, 2 * H2 * NM), np.float32)
    for p in range(NM):
        for q in range(H2):
            for o in range(H1):
                W2L[_hrow(p, o, 0), _h2row(p, q, 0)] += W2r[p, q, o]
                W2L[_hrow(p, o, 1), _h2row(p, q, 0)] -= W2i[p, q, o]
                W2L[_hrow(p, o, 0), _h2row(p, q, 1)] += W2i[p, q, o]
                W2L[_hrow(p, o, 1), _h2row(p, q, 1)] += W2r[p, q, o]
    # W3 lhsT [40, 4]: out rows (comp, p): [re_p0, re_p1, im_p0, im_p1]; 1/NM folded
    W3L = np.zeros((2 * H2 * NM, 4), np.float32)
    s = 1.0 / NM
    for p in range(NM):
        for q in range(H2):
            W3L[_h2row(p, q, 0), 0 + p] += W3r[p, 0, q] * s
            W3L[_h2row(p, q, 1), 0 + p] -= W3i[p, 0, q] * s
            W3L[_h2row(p, q, 0), 2 + p] += W3i[p, 0, q] * s
            W3L[_h2row(p, q, 1), 2 + p] += W3r[p, 0, q] * s
    return {"RWP": RWP, "W2L": W2L, "W3L": W3L}


# ---------------------------------------------------------------------------
def build_nc(bcore=BCORE, mm_dtype_name="float32r", nb=NB, lrelu_mode="act"):
    """Build the Bass program for one core processing `bcore` samples."""
    import concourse.bass as bass
    import concourse.bacc as bacc
    import concourse.mybir as mybir
    from concourse.tile import TileContext
    import bass_rust

    nchunk = bcore // nb
    assert nchunk * nb == bcore
    grp = 4 if nchunk % 4 == 0 else 1
    f32 = mybir.dt.float32
    mmdt = getattr(mybir.dt, mm_dtype_name)
    AF = bass_rust.ActivationFunctionType
    OP = mybir.AluOpType

    nc = bacc.Bacc(None, target_bir_lowering=False, debug=False)
    xTr = nc.declare_dram_parameter("xTr", [ROWS + 2, bcore], f32, isOutput=False)
    xTi = nc.declare_dram_parameter("xTi", [ROWS, bcore], f32, isOutput=False)
    selD = nc.declare_dram_parameter("SEL", [ROWS, 2 * NSTACK], f32, isOutput=False)
    xrwD = nc.declare_dram_parameter("XREPW", [ROWS, NO], f32, isOutput=False)
    finD = nc.declare_dram_parameter("FINW", [NO, 16], f32, isOutput=False)
    rwD = nc.declare_dram_parameter("RWP", [3, 128, 4 * NO], f32, isOutput=False)
    w2D = nc.declare_dram_parameter("W2L", [8, 40], f32, isOutput=False)
    w3D = nc.declare_dram_parameter("W3L", [40, 4], f32, isOutput=False)
    outs_d = [nc.declare_dram_parameter(f"out{j}", [4, grp * nb], f32, isOutput=True)
              for j in range(max(1, nchunk // grp))]

    use_r = mm_dtype_name != "float32"

    def r_(ap):   # matmul operands are already declared in the matmul dtype
        return ap

    with TileContext(nc) as tc:
        with (
            tc.tile_pool(name="consts", bufs=1) as cp,
            tc.tile_pool(name="xt", bufs=3) as xp,
            tc.tile_pool(name="g", bufs=2) as gp,
            tc.tile_pool(name="tmp", bufs=2) as tp,
            tc.tile_pool(name="tt", bufs=2) as ttp,
            tc.tile_pool(name="small", bufs=3) as sp,
            tc.tile_pool(name="psum", bufs=4, space="PSUM") as pp,
        ):
            def const_tile(src_ap, name):
                t32 = cp.tile(list(src_ap.shape), f32, name=name + "_32")
                nc.gpsimd.dma_start(out=t32[:], in_=src_ap)
                if not use_r:
                    return t32
                tr = cp.tile(list(src_ap.shape), mmdt, name=name)
                nc.vector.tensor_copy(tr[:], t32[:])
                return tr

            sel_sb = const_tile(selD[:], "sel")
            xrw_sb = const_tile(xrwD[:], "xrw")
            fin_sb = const_tile(finD[:], "fin")
            rw_sb = [const_tile(rwD[k], f"rw{k}") for k in range(3)]
            w2_sb = const_tile(w2D[:], "w2")
            w3_sb = const_tile(w3D[:], "w3")

            for c in range(nchunk):
                cs = slice(c * nb, (c + 1) * nb)
                # ---- load transposed x chunk [82, nb]
                # HWDGE f32 loads + ACT bf16 cast: keeps DMA-trigger ucode off the
                # Pool queue, which runs the G/T adds
                xr32 = xp.tile([98, nb], f32, tag="xr32", bufs=3)
                xi32 = xp.tile([ROWS, nb], f32, tag="xi32", bufs=3)
                nc.sync.dma_start(out=xr32[0:ROWS], in_=xTr[0:ROWS, cs])
                nc.sync.dma_start(out=xr32[96:98], in_=xTr[ROWS:ROWS + 2, cs])
                nc.sync.dma_start(out=xi32[:], in_=xTi[:, cs])
                xr = xp.tile([98, nb], mmdt, tag="xr", bufs=3)
                xi = xp.tile([ROWS, nb], mmdt, tag="xi", bufs=3)
                nc.scalar.copy(xr[0:ROWS], xr32[0:ROWS])
                nc.scalar.copy(xr[96:98], xr32[96:98])
                nc.scalar.copy(xi[:], xi32[:])

                # ---- gather matmuls + G products per psplit
                g_tiles = []
                for k, (r0, rk) in enumerate(KSPLITS):
                    pa_r = pp.tile([128, nb], f32, tag="pp")
                    pa_i = pp.tile([128, nb], f32, tag="pp")
                    pc_r = pp.tile([128, nb], f32, tag="pp")
                    pc_i = pp.tile([128, nb], f32, tag="pp")
                    a_sl = sel_sb[:, r0:r0 + rk]
                    c_sl = sel_sb[:, NSTACK + r0:NSTACK + r0 + rk]
                    # C-side first, copy each to SBUF right after its matmul so
                    # ACT feeds the DVE products with minimal latency
                    cr_s = tp.tile([128, nb], f32, tag="crs", bufs=4)
                    ci_s = tp.tile([128, nb], f32, tag="cis", bufs=4)
                    nc.tensor.matmul(pc_r[:rk], r_(c_sl), r_(xr[:ROWS]), start=True, stop=True)
                    nc.scalar.copy(cr_s[:rk], pc_r[:rk])
                    nc.tensor.matmul(pc_i[:rk], r_(c_sl), r_(xi[:]), start=True, stop=True)
                    nc.scalar.copy(ci_s[:rk], pc_i[:rk])
                    nc.tensor.matmul(pa_r[:rk], r_(a_sl), r_(xr[:ROWS]), start=True, stop=True)
                    nc.tensor.matmul(pa_i[:rk], r_(a_sl), r_(xi[:]), start=True, stop=True)
                    # G = A * conj(C)
                    t0 = tp.tile([128, nb], f32, tag="t0", bufs=4)
                    t1 = tp.tile([128, nb], f32, tag="t1", bufs=4)
                    gr = gp.tile([128, nb], mmdt, tag=f"gr{k}")
                    gi = gp.tile([128, nb], mmdt, tag=f"gi{k}")
                    nc.vector.tensor_tensor(t0[:rk], pa_r[:rk], cr_s[:rk], op=OP.mult)
                    nc.vector.tensor_tensor(t1[:rk], pa_i[:rk], ci_s[:rk], op=OP.mult)
                    nc.gpsimd.tensor_tensor(gr[:rk], t0[:rk], t1[:rk], op=OP.add)
                    nc.vector.tensor_tensor(t0[:rk], pa_i[:rk], cr_s[:rk], op=OP.mult)
                    nc.vector.tensor_tensor(t1[:rk], pa_r[:rk], ci_s[:rk], op=OP.mult)
                    nc.gpsimd.tensor_tensor(gi[:rk], t0[:rk], t1[:rk], op=OP.subtract)
                    g_tiles.append((gr, gi))

                # ---- R matmuls: accumulate over 3 psplits x (Gr, Gi)
                p_rre = pp.tile([128, nb], f32, tag="racc", bufs=2)
                p_rim = pp.tile([128, nb], f32, tag="racc", bufs=2)
                for k, (r0, rk) in enumerate(KSPLITS):
                    gr, gi = g_tiles[k]
                    rw = rw_sb[k]
                    nc.tensor.matmul(p_rre[:NO], r_(rw[:rk, 0:NO]), r_(gr[:rk]),
                                     start=(k == 0), stop=False)
                    nc.tensor.matmul(p_rre[:NO], r_(rw[:rk, NO:2 * NO]), r_(gi[:rk]),
                                     start=False, stop=(k == 2))
                    nc.tensor.matmul(p_rim[:NO], r_(rw[:rk, 2 * NO:3 * NO]), r_(gr[:rk]),
                                     start=(k == 0), stop=False)
                    nc.tensor.matmul(p_rim[:NO], r_(rw[:rk, 3 * NO:4 * NO]), r_(gi[:rk]),
                                     start=False, stop=(k == 2))

                # ---- xrep + center matmuls
                p_xr = pp.tile([128, nb], f32, tag="misc", bufs=2)
                p_xi = pp.tile([128, nb], f32, tag="misc", bufs=2)
                nc.tensor.matmul(p_xr[:NO], r_(xrw_sb[:]), r_(xr[:ROWS]), start=True, stop=True)
                nc.tensor.matmul(p_xi[:NO], r_(xrw_sb[:]), r_(xi[:]), start=True, stop=True)

                # ---- T products (complex xrep * R); R copied to SBUF first
                rre_s = tp.tile([NO, nb], f32, tag="rres")
                rim_s = tp.tile([NO, nb], f32, tag="rims")
                nc.scalar.copy(rre_s[:], p_rre[:NO])
                nc.scalar.copy(rim_s[:], p_rim[:NO])
                u0 = tp.tile([128, nb], f32, tag="u0")
                u1 = tp.tile([128, nb], f32, tag="u1")
                t_re = ttp.tile([NO, nb], mmdt, tag="tre")
                t_im = ttp.tile([NO, nb], mmdt, tag="tim")
                nc.vector.tensor_tensor(u0[:NO], p_xr[:NO], rre_s[:], op=OP.mult)
                nc.vector.tensor_tensor(u1[:NO], p_xi[:NO], rim_s[:], op=OP.mult)
                nc.gpsimd.tensor_tensor(t_re[:], u0[:NO], u1[:NO], op=OP.subtract)
                nc.vector.tensor_tensor(u0[:NO], p_xr[:NO], rim_s[:], op=OP.mult)
                nc.vector.tensor_tensor(u1[:NO], p_xi[:NO], rre_s[:], op=OP.mult)
                nc.gpsimd.tensor_tensor(t_im[:], u0[:NO], u1[:NO], op=OP.add)

                # ---- final contraction -> h1 [8, nb]
                p_h1 = pp.tile([128, nb], f32, tag="misc", bufs=2)
                nc.tensor.matmul(p_h1[:8], r_(fin_sb[:, 0:8]), r_(t_re[:]), start=True, stop=False)
                nc.tensor.matmul(p_h1[:8], r_(fin_sb[:, 8:16]), r_(t_im[:]), start=False, stop=True)

                # ---- MLP tail
                def lrelu(dst, src, rows):
                    if lrelu_mode == "act":
                        nc.scalar.activation(dst[:rows], src[:rows], AF.Lrelu, alpha=SLOPE)
                    else:
                        nc.vector.tensor_scalar_mul(dst[:rows], src[:rows], SLOPE)
                        nc.vector.tensor_tensor(dst[:rows], dst[:rows], src[:rows], op=OP.max)

                h1s = sp.tile([8, nb], mmdt, tag="h1s")
                lrelu(h1s, p_h1, 8)
                p_h2 = pp.tile([128, nb], f32, tag="misc", bufs=2)
                nc.tensor.matmul(p_h2[:40], r_(w2_sb[:]), r_(h1s[:8]), start=True, stop=True)
                h2s = sp.tile([40, nb], mmdt, tag="h2s")
                lrelu(h2s, p_h2, 40)
                p_ere = pp.tile([128, nb], f32, tag="misc", bufs=2)
                p_eim = pp.tile([128, nb], f32, tag="misc", bufs=2)
                nc.tensor.matmul(p_ere[:2], r_(w3_sb[:, 0:2]), r_(h2s[:]), start=True, stop=True)
                nc.tensor.matmul(p_eim[:2], r_(w3_sb[:, 2:4]), r_(h2s[:]), start=True, stop=True)

                # ---- P = 10^(t/10) = exp(t * ln10/10); out = center + E*P
                pex = sp.tile([2, nb], f32, tag="pex")
                nc.scalar.activation(pex[:], xr[96:98], AF.Exp,
                                     scale=float(np.log(10.0) / 10.0))
                if c % grp == 0:
                    cs4 = slice(c * nb, (c + grp) * nb)
                    o_re2 = sp.tile([2, grp * nb], f32, tag="ore", bufs=max(1, nchunk // grp), name="o_re2")
                    o_im2 = sp.tile([2, grp * nb], f32, tag="oim", bufs=max(1, nchunk // grp), name="o_im2")
                    nc.sync.dma_start(out=o_re2[:], in_=xTr[2 * LH:2 * LH + 2, cs4])
                    nc.sync.dma_start(out=o_im2[:], in_=xTi[2 * LH:2 * LH + 2, cs4])
                    chunk_pair = (o_re2, o_im2)
                half = (c % grp) * nb
                o_re = chunk_pair[0][:, half:half + nb]
                o_im = chunk_pair[1][:, half:half + nb]
                ep_r = tp.tile([2, nb], f32, tag="epr")
                ep_i = tp.tile([2, nb], f32, tag="epi")
                nc.vector.tensor_tensor(ep_r[:], p_ere[:2], pex[:], op=OP.mult)
                nc.vector.tensor_tensor(ep_i[:], p_eim[:2], pex[:], op=OP.mult)
                nc.gpsimd.tensor_tensor(o_re, ep_r[:], o_re, op=OP.add)
                nc.gpsimd.tensor_tensor(o_im, ep_i[:], o_im, op=OP.add)
                if c % grp == grp - 1:
                    od = outs_d[c // grp]
                    nc.sync.dma_start(out=od[0:2, :], in_=chunk_pair[0][:])
                    nc.sync.dma_start(out=od[2:4, :], in_=chunk_pair[1][:])
    nc.compile()
    return nc


def _prep_core_inputs(inputs, static, folded):
    """Shard + lay out inputs per core. Returns list of in_maps."""
    xr = np.ascontiguousarray(inputs["x_real"]).reshape(BATCH, ROWS)
    xi = np.ascontiguousarray(inputs["x_imag"]).reshape(BATCH, ROWS)
    t0 = np.ascontiguousarray(inputs["task_info"][:, 0])
    shared = {
        "SEL": static["SEL"], "XREPW": static["XREPW"],
        "FINW": static["FINW"], "RWP": folded["RWP"], "W2L": folded["W2L"],
        "W3L": folded["W3L"],
    }
    in_maps = []
    for c in range(NCORES):
        s = slice(c * BCORE, (c + 1) * BCORE)
        m = dict(shared)
        m["xTr"] = np.ascontiguousarray(
            np.concatenate([xr[s].T, np.broadcast_to(t0[s][None, :], (2, BCORE))], axis=0))
        m["xTi"] = np.ascontiguousarray(xi[s].T)
        in_maps.append(m)
    return in_maps


_CACHE = {}
KERNEL_MM_DTYPE = "bfloat16"   # matmul operand dtype: bfloat16 | float32r | float32


def kernel(**inputs):
    from concourse.bass_utils import run_bass_kernel_spmd

    static = build_static()
    folded = fold_weights(
        np.asarray(inputs["W1_real"]), np.asarray(inputs["W1_imag"]),
        np.asarray(inputs["W2_real"]), np.asarray(inputs["W2_imag"]),
        np.asarray(inputs["W3_real"]), np.asarray(inputs["W3_imag"]),
    )
    if "nc" not in _CACHE:
        _CACHE["nc"] = build_nc(mm_dtype_name=KERNEL_MM_DTYPE)
    nc = _CACHE["nc"]
    in_maps = _prep_core_inputs(inputs, static, folded)
    res = run_bass_kernel_spmd(nc, in_maps, list(range(NCORES)))
    nseg = NCHUNK // 4
    out = np.empty((BATCH, NM, 2), np.float32)
    for c in range(NCORES):
        o4 = np.concatenate([res.results[c][f"out{j}"] for j in range(nseg)], axis=1)
        s = slice(c * BCORE, (c + 1) * BCORE)
        out[s, 0, 0] = o4[0]
        out[s, 1, 0] = o4[1]
        out[s, 0, 1] = o4[2]
        out[s, 1, 1] = o4[3]
    return out



# revision 6
# speedup vs baseline: 1.7587x; 1.7587x over previous
"""EqPBCNN (perturbation-based nonlinearity compensation NN) Trainium2 Bass kernel.

Data-parallel over 8 NeuronCores: batch 65536 -> 8192 per core.

Math (per sample):
  G_(a,b) = sum_q x[a,q] * conj(x[b,q])      (pol-independent; pairs (a,b)=(n+L, m+n+L))
  h1[p,o] = sum_m x[m,p] * R[p,o,m],  R = sum_n W1'[p,o,(m,n)] * G
  h2 = CLrelu(h1) @ W2^T; E = CLrelu(h2) @ W3^T
  out = x[center,p] + E * 10^(task0/10)/2

v2 design: conjugate-canonical pairs (148 of 175; G_(b,a) = conj(G_(a,b)) folded
into the R weights with signs). Host pre-gathers the pair stacks into DRAM:
  SAr/SAi/SBr/SBi [296, B/8] bf16   rows = (q, off-diag pairs(127), q, diag(21))
  XPr/XPi [100, B/8] bf16           rows = (o, mi, p) x-replica for the T product
Device pipeline per chunk (NS=1024 cols):
  DVE products P1=SAr*SBr, P2=SAi*SBi (296 rows), P3=SAi*SBr, P4=SAr*SBi (254 rows)
  Gim = P3 - P4 on DVE; Gre = P1 + P2 folded into PE (P1,P2 fed separately)
  R matmuls (PE, bf16) -> Rre/Rim [100, NS] PSUM -> ACT copy to SBUF bf16
  T products U=XP*R (DVE bf16) -> final matmul (PE) -> h1 [8, NS]
  ACT lrelu / W2 / lrelu / W3 -> E [4, NS]; ACT exp for P; DVE E*P; Pool residual.
"""
import numpy as np
import ml_dtypes

BF16 = ml_dtypes.bfloat16

# ---------------- problem constants (hardcoded; must match reference) -------
BATCH = 65536
MT, LH = 41, 20          # filter taps, half window
NM = 2                   # modes / polarizations
H1, H2 = 2, 10
SLOPE = 0.01
NCORES = 8
BCORE = BATCH // NCORES  # 8192
NS = 1024               # columns per compute chunk (mega == chunk)
NCHUNK = BCORE // NS     # 8

# ---------------- triplet / canonical-pair tables ---------------------------
_idx = [(m, n) for m in range(-LH, LH + 1) for n in range(-LH, LH + 1)
        if abs(m * n) <= LH and abs(m + n) <= LH and n >= m]
H = len(_idx)            # 175
SYM = np.where(np.array([m for m, n in _idx]) != np.array([n for m, n in _idx]),
               2.0, 1.0).astype(np.float32)
M_VALS = sorted(set(m for m, n in _idx))     # 25 distinct m values
NMV = len(M_VALS)
M_POS = {m: i for i, m in enumerate(M_VALS)}
NO = H1 * NMV * NM       # 100 rows of R/T space: (o, mi, p)

# canonical pairs: key (a,b) a<=b; triplet h -> (pair index, Gim sign)
_ckeys = {}
_tripmap = []
for (m, n) in _idx:
    a, b = n + LH, m + n + LH
    key, s = ((a, b), 1.0) if a <= b else ((b, a), -1.0)
    _ckeys[key] = None
    _tripmap.append((key, s))
POFF = sorted([k for k in _ckeys if k[0] < k[1]], key=lambda k: (k[1] - k[0], k[0]))
PDIAG = sorted([k for k in _ckeys if k[0] == k[1]])
NOFF, NDIAG = len(POFF), len(PDIAG)          # 127, 21
NPAIR = NOFF + NDIAG                          # 148
# stack rows: q0-off(127), q1-off(127), q0-diag(21), q1-diag(21)
NROWS_RE = 2 * NPAIR                          # 296 (P1/P2/Gre rows)
NROWS_IM = 2 * NOFF                           # 254 (P3/P4/Gim rows)
_pairpos = {}
for i, k in enumerate(POFF):
    _pairpos[k] = ('off', i)
for i, k in enumerate(PDIAG):
    _pairpos[k] = ('diag', i)


def _stack_row(kind, i, q):
    return q * NOFF + i if kind == 'off' else NROWS_IM + q * NDIAG + i


def _orow(o, mi, p):
    return (o * NMV + mi) * NM + p


def _hrow(p, o, comp):
    return (p * H1 + o) * 2 + comp


def _h2row(p, q, comp):
    return (p * H2 + q) * 2 + comp


# split boundaries of the 296-row stacks
KSP_RE = [(0, 128), (128, 128), (256, NROWS_RE - 256)]   # 128,128,40
KSP_IM = [(0, 128), (128, NROWS_IM - 128)]               # 128,126


def build_static():
    """Gather row tables (host side) + final/contraction constants."""
    # source rows within xq82 [2*41, BCORE] (rows = q*41 + tap)
    a_src = np.zeros(NROWS_RE, np.int64)
    b_src = np.zeros(NROWS_RE, np.int64)
    for key in POFF + PDIAG:
        kind, i = _pairpos[key]
        a, b = key
        for q in range(NM):
            r = _stack_row(kind, i, q)
            a_src[r] = q * MT + a
            b_src[r] = q * MT + b
    # xrep rows (o, mi, p) -> tap m+L, pol p
    xp_src = np.zeros(NO, np.int64)
    for o in range(H1):
        for mi, mv in enumerate(M_VALS):
            for p in range(NM):
                xp_src[_orow(o, mi, p)] = p * MT + (mv + LH)
    # final contraction [100, 32]: 8-col groups for U1(+re) U2(-re) U3(+im) U4(+im)
    FINW = np.zeros((NO, 32), np.float32)
    for o in range(H1):
        for mi in range(NMV):
            for p in range(NM):
                r = _orow(o, mi, p)
                FINW[r, 0 + _hrow(p, o, 0)] = 1.0
                FINW[r, 8 + _hrow(p, o, 0)] = -1.0
                FINW[r, 16 + _hrow(p, o, 1)] = 1.0
                FINW[r, 24 + _hrow(p, o, 1)] = 1.0
    return {"a_src": a_src, "b_src": b_src, "xp_src": xp_src, "FINW": FINW}


def fold_weights(W1r, W1i, W2r, W2i, W3r, W3i):
    """Fold W1 (with SYM, pol-sum dup, conj-pair signs) into R-matmul lhsT."""
    Wr = W1r * SYM[None, None, :]   # [p, o, h]
    Wi = W1i * SYM[None, None, :]
    # WG [296, 200]: cols 0:100 -> Rre (+Wr), 100:200 -> Rim (+Wi); fed by P1 AND P2
    WG = np.zeros((NROWS_RE, 2 * NO), np.float32)
    # WI [254, 200]: Gim rows; cols 0:100 -> Rre (-s*Wi), 100:200 -> Rim (+s*Wr)
    WI = np.zeros((NROWS_IM, 2 * NO), np.float32)
    for h, (mn, (key, s)) in enumerate(zip(_idx, _tripmap)):
        m, n = mn
        kind, i = _pairpos[key]
        mi = M_POS[m]
        for p in range(NM):
            for o in range(H1):
                c = _orow(o, mi, p)
                for q in range(NM):
                    r = _stack_row(kind, i, q)
                    WG[r, c] += Wr[p, o, h]
                    WG[r, NO + c] += Wi[p, o, h]
                    if kind == 'off':
                        WI[r, c] += -s * Wi[p, o, h]
                        WI[r, NO + c] += s * Wr[p, o, h]
    WGP = np.zeros((3, 128, 2 * NO), np.float32)
    for k, (r0, rk) in enumerate(KSP_RE):
        WGP[k, :rk, :] = WG[r0:r0 + rk, :]
    WIP = np.zeros((2, 128, 2 * NO), np.float32)
    for k, (r0, rk) in enumerate(KSP_IM):
        WIP[k, :rk, :] = WI[r0:r0 + rk, :]
    # W2 lhsT [8, 40] on h1 rows (p,o,comp)
    W2L = np.zeros((8, 2 * H2 * NM), np.float32)
    for p in range(NM):
        for q in range(H2):
            for o in range(H1):
                W2L[_hrow(p, o, 0), _h2row(p, q, 0)] += W2r[p, q, o]
                W2L[_hrow(p, o, 1), _h2row(p, q, 0)] -= W2i[p, q, o]
                W2L[_hrow(p, o, 0), _h2row(p, q, 1)] += W2i[p, q, o]
                W2L[_hrow(p, o, 1), _h2row(p, q, 1)] += W2r[p, q, o]
    # W3 lhsT [40, 4]: out rows [re_p0, re_p1, im_p0, im_p1]; 1/NM folded
    W3L = np.zeros((2 * H2 * NM, 4), np.float32)
    s3 = 1.0 / NM
    for p in range(NM):
        for q in range(H2):
            W3L[_h2row(p, q, 0), 0 + p] += W3r[p, 0, q] * s3
            W3L[_h2row(p, q, 1), 0 + p] -= W3i[p, 0, q] * s3
            W3L[_h2row(p, q, 0), 2 + p] += W3i[p, 0, q] * s3
            W3L[_h2row(p, q, 1), 2 + p] += W3r[p, 0, q] * s3
    return {"WGP": WGP, "WIP": WIP, "W2L": W2L, "W3L": W3L}


# ---------------------------------------------------------------------------
def build_nc(bcore=BCORE, lrelu_mode="act"):
    """Build the Bass program for one core processing `bcore` samples."""
    import concourse.bass as bass
    import concourse.bacc as bacc
    import concourse.mybir as mybir
    from concourse.tile import TileContext
    import bass_rust

    nchunk = bcore // NS
    assert nchunk * NS == bcore
    f32 = mybir.dt.float32
    bf16 = mybir.dt.bfloat16
    AF = bass_rust.ActivationFunctionType
    OP = mybir.AluOpType

    nc = bacc.Bacc(None, target_bir_lowering=False, debug=False)
    saR = nc.declare_dram_parameter("SAr", [NROWS_RE, bcore], bf16, isOutput=False)
    saI = nc.declare_dram_parameter("SAi", [NROWS_RE, bcore], bf16, isOutput=False)
    sbR = nc.declare_dram_parameter("SBr", [NROWS_RE, bcore], bf16, isOutput=False)
    sbI = nc.declare_dram_parameter("SBi", [NROWS_RE, bcore], bf16, isOutput=False)
    xpR = nc.declare_dram_parameter("XPr", [NO, bcore], bf16, isOutput=False)
    xpI = nc.declare_dram_parameter("XPi", [NO, bcore], bf16, isOutput=False)
    tk4 = nc.declare_dram_parameter("TASK4", [4, bcore], f32, isOutput=False)
    ctrD = nc.declare_dram_parameter("CTR", [4, bcore], f32, isOutput=False)
    wgD = nc.declare_dram_parameter("WGP", [3, 128, 2 * NO], f32, isOutput=False)
    wiD = nc.declare_dram_parameter("WIP", [2, 128, 2 * NO], f32, isOutput=False)
    finD = nc.declare_dram_parameter("FINW", [NO, 32], f32, isOutput=False)
    w2D = nc.declare_dram_parameter("W2L", [8, 40], f32, isOutput=False)
    w3D = nc.declare_dram_parameter("W3L", [40, 4], f32, isOutput=False)
    outD = nc.declare_dram_parameter("OUT", [4, bcore], f32, isOutput=True)

    with TileContext(nc) as tc:
        with (
            tc.tile_pool(name="consts", bufs=1) as cp,
            tc.tile_pool(name="mega", bufs=2) as mp,
            tc.tile_pool(name="small", bufs=2) as sp,
            tc.tile_pool(name="prod", bufs=2) as up,
            tc.tile_pool(name="tt", bufs=2) as tp,
            tc.tile_pool(name="psum", bufs=4, space="PSUM") as pp,
        ):
            def const_tile(src_ap, name):
                t32 = cp.tile(list(src_ap.shape), f32, name=name + "_32")
                nc.gpsimd.dma_start(out=t32[:], in_=src_ap)
                tr = cp.tile(list(src_ap.shape), bf16, name=name)
                nc.vector.tensor_copy(tr[:], t32[:])
                return tr

            def mm2(out_ap, lhsT, rhs, start, stop):
                # PSUM bank = 512 f32 cols; split wide matmuls into halves
                h = NS // 2
                nc.tensor.matmul(out_ap[:, 0:h], lhsT, rhs[:, 0:h], start=start, stop=stop)
                nc.tensor.matmul(out_ap[:, h:NS], lhsT, rhs[:, h:NS], start=start, stop=stop)

            wg_sb = [const_tile(wgD[k], f"wg{k}") for k in range(3)]
            wi_sb = [const_tile(wiD[k], f"wi{k}") for k in range(2)]
            fin_sb = const_tile(finD[:], "fin")
            w2_sb = const_tile(w2D[:], "w2")
            w3_sb = const_tile(w3D[:], "w3")

            for c in range(nchunk):
                cs = slice(c * NS, (c + 1) * NS)
                # ---- chunk loads (bf16 stacks + f32 task/center)
                sa_r, sa_i, sb_r, sb_i = [], [], [], []
                for k, (r0, rk) in enumerate(KSP_RE):
                    for nm_, src, lst in (
                        (f"sar{k}", saR, sa_r), (f"sai{k}", saI, sa_i),
                        (f"sbr{k}", sbR, sb_r), (f"sbi{k}", sbI, sb_i)):
                        t = mp.tile([rk, NS], bf16, tag=nm_)
                        nc.sync.dma_start(out=t[:], in_=src[r0:r0 + rk, cs])
                        lst.append(t)
                xp_r = mp.tile([NO, NS], bf16, tag="xpr")
                xp_i = mp.tile([NO, NS], bf16, tag="xpi")
                nc.sync.dma_start(out=xp_r[:], in_=xpR[:, cs])
                nc.sync.dma_start(out=xp_i[:], in_=xpI[:, cs])
                tk_t = sp.tile([4, NS], f32, tag="tk")
                ct_t = sp.tile([4, NS], f32, tag="ct")
                nc.sync.dma_start(out=tk_t[:], in_=tk4[:, cs])
                nc.sync.dma_start(out=ct_t[:], in_=ctrD[:, cs])
                # P = 10^(t/10) = exp(t * ln10/10)
                pex = sp.tile([4, NS], f32, tag="pex")
                nc.scalar.activation(pex[:], tk_t[:], AF.Exp,
                                     scale=float(np.log(10.0) / 10.0))

                # ---- DVE products (bf16 SBUF x SBUF, 2x mode)
                p1, p2, p3 = [], [], []
                for k, (r0, rk) in enumerate(KSP_RE):
                    t1 = up.tile([rk, NS], bf16, tag=f"p1_{k}")
                    t2 = up.tile([rk, NS], bf16, tag=f"p2_{k}")
                    nc.vector.tensor_tensor(t1[:], sa_r[k][:], sb_r[k][:], op=OP.mult)
                    nc.vector.tensor_tensor(t2[:], sa_i[k][:], sb_i[k][:], op=OP.mult)
                    p1.append(t1)
                    p2.append(t2)
                for k, (r0, rk) in enumerate(KSP_IM):
                    t3 = up.tile([rk, NS], bf16, tag=f"p3_{k}")
                    t4 = up.tile([rk, NS], bf16, tag=f"p4_{k}")
                    nc.vector.tensor_tensor(t3[:], sa_i[k][:rk, :], sb_r[k][:rk, :], op=OP.mult)
                    nc.vector.tensor_tensor(t4[:], sa_r[k][:rk, :], sb_i[k][:rk, :], op=OP.mult)
                    # Gim = P3 - P4 in place over t3
                    nc.vector.tensor_tensor(t3[:], t3[:], t4[:], op=OP.subtract)
                    p3.append(t3)

                # ---- R matmuls: Rre/Rim accumulate P1,P2 (x3) and Gim (x2)
                p_rre = pp.tile([128, NS], f32, tag="ps")
                p_rim = pp.tile([128, NS], f32, tag="ps")
                for k, (r0, rk) in enumerate(KSP_RE):
                    wg = wg_sb[k]
                    mm2(p_rre[:NO], wg[:rk, 0:NO], p1[k], start=(k == 0), stop=False)
                    mm2(p_rre[:NO], wg[:rk, 0:NO], p2[k], start=False, stop=False)
                    mm2(p_rim[:NO], wg[:rk, NO:2 * NO], p1[k], start=(k == 0), stop=False)
                    mm2(p_rim[:NO], wg[:rk, NO:2 * NO], p2[k], start=False, stop=False)
                for k, (r0, rk) in enumerate(KSP_IM):
                    wi = wi_sb[k]
                    mm2(p_rre[:NO], wi[:rk, 0:NO], p3[k], start=False, stop=(k == 1))
                    mm2(p_rim[:NO], wi[:rk, NO:2 * NO], p3[k], start=False, stop=(k == 1))

                # ---- T products: U = XP * R (R copied to SBUF bf16 first)
                rre_s = tp.tile([NO, NS], bf16, tag="rres")
                rim_s = tp.tile([NO, NS], bf16, tag="rims")
                nc.scalar.copy(rre_s[:], p_rre[:NO])
                nc.scalar.copy(rim_s[:], p_rim[:NO])
                u1 = tp.tile([NO, NS], bf16, tag="u1")
                u2 = tp.tile([NO, NS], bf16, tag="u2")
                u3 = tp.tile([NO, NS], bf16, tag="u3")
                u4 = tp.tile([NO, NS], bf16, tag="u4")
                nc.vector.tensor_tensor(u1[:], xp_r[:], rre_s[:], op=OP.mult)
                nc.vector.tensor_tensor(u2[:], xp_i[:], rim_s[:], op=OP.mult)
                nc.vector.tensor_tensor(u3[:], xp_r[:], rim_s[:], op=OP.mult)
                nc.vector.tensor_tensor(u4[:], xp_i[:], rre_s[:], op=OP.mult)

                # ---- final contraction -> h1 [8, NS]
                p_h1 = pp.tile([128, NS], f32, tag="ps")
                mm2(p_h1[:8], fin_sb[:, 0:8], u1, start=True, stop=False)
                mm2(p_h1[:8], fin_sb[:, 8:16], u2, start=False, stop=False)
                mm2(p_h1[:8], fin_sb[:, 16:24], u3, start=False, stop=False)
                mm2(p_h1[:8], fin_sb[:, 24:32], u4, start=False, stop=True)

                # ---- MLP tail
                def lrelu(dst, src_ap):
                    if lrelu_mode == "act":
                        nc.scalar.activation(dst, src_ap, AF.Lrelu, alpha=SLOPE)
                    else:
                        nc.vector.tensor_scalar_mul(dst, src_ap, SLOPE)
                        nc.vector.tensor_tensor(dst, dst, src_ap, op=OP.max)

                h1s = tp.tile([8, NS], bf16, tag="h1s")
                lrelu(h1s[:], p_h1[:8])
                p_h2 = pp.tile([128, NS], f32, tag="ps")
                mm2(p_h2[:40], w2_sb[:], h1s, start=True, stop=True)
                h2s = tp.tile([40, NS], bf16, tag="h2s")
                lrelu(h2s[:], p_h2[:40])
                p_e = pp.tile([128, NS], f32, tag="ps")
                mm2(p_e[:4], w3_sb[:], h2s, start=True, stop=True)

                # ---- out = CTR + E*P
                ep = tp.tile([4, NS], f32, tag="ep")
                outm = sp.tile([4, NS], f32, tag="outm")
                nc.vector.tensor_tensor(ep[:], p_e[:4], pex[:], op=OP.mult)
                nc.gpsimd.tensor_tensor(outm[:], ep[:], ct_t[:], op=OP.add)
                nc.sync.dma_start(out=outD[:, cs], in_=outm[:])
    nc.compile()
    return nc


# ---------------------------------------------------------------------------
def _prep_core_inputs(inputs, static, folded):
    """Host-side gather + shard. Returns list of per-core in_maps."""
    xr = np.asarray(inputs["x_real"])     # [B, 41, 2]
    xi = np.asarray(inputs["x_imag"])
    t0 = np.ascontiguousarray(np.asarray(inputs["task_info"])[:, 0])
    # xq82 rows = q*41 + tap
    xrq = np.ascontiguousarray(xr.transpose(2, 1, 0).reshape(2 * MT, BATCH))
    xiq = np.ascontiguousarray(xi.transpose(2, 1, 0).reshape(2 * MT, BATCH))
    a_src, b_src, xp_src = static["a_src"], static["b_src"], static["xp_src"]
    SAr = xrq[a_src].astype(BF16)
    SAi = xiq[a_src].astype(BF16)
    SBr = xrq[b_src].astype(BF16)
    SBi = xiq[b_src].astype(BF16)
    XPr = xrq[xp_src].astype(BF16)
    XPi = xiq[xp_src].astype(BF16)
    TASK4 = np.broadcast_to(t0[None, :], (4, BATCH))
    CTR = np.stack([xrq[LH], xrq[MT + LH], xiq[LH], xiq[MT + LH]], axis=0)
    shared = {"WGP": folded["WGP"], "WIP": folded["WIP"], "FINW": static["FINW"],
              "W2L": folded["W2L"], "W3L": folded["W3L"]}
    in_maps = []
    for c in range(NCORES):
        s = slice(c * BCORE, (c + 1) * BCORE)
        m = dict(shared)
        m["SAr"] = np.ascontiguousarray(SAr[:, s])
        m["SAi"] = np.ascontiguousarray(SAi[:, s])
        m["SBr"] = np.ascontiguousarray(SBr[:, s])
        m["SBi"] = np.ascontiguousarray(SBi[:, s])
        m["XPr"] = np.ascontiguousarray(XPr[:, s])
        m["XPi"] = np.ascontiguousarray(XPi[:, s])
        m["TASK4"] = np.ascontiguousarray(TASK4[:, s])
        m["CTR"] = np.ascontiguousarray(CTR[:, s])
        in_maps.append(m)
    return in_maps


_CACHE = {}


def kernel(**inputs):
    from concourse.bass_utils import run_bass_kernel_spmd

    static = build_static()
    folded = fold_weights(
        np.asarray(inputs["W1_real"]), np.asarray(inputs["W1_imag"]),
        np.asarray(inputs["W2_real"]), np.asarray(inputs["W2_imag"]),
        np.asarray(inputs["W3_real"]), np.asarray(inputs["W3_imag"]),
    )
    if "nc" not in _CACHE:
        _CACHE["nc"] = build_nc()
    nc = _CACHE["nc"]
    in_maps = _prep_core_inputs(inputs, static, folded)
    res = run_bass_kernel_spmd(nc, in_maps, list(range(NCORES)))
    out = np.empty((BATCH, NM, 2), np.float32)
    for c in range(NCORES):
        o4 = res.results[c]["OUT"]
        s = slice(c * BCORE, (c + 1) * BCORE)
        out[s, 0, 0] = o4[0]
        out[s, 1, 0] = o4[1]
        out[s, 0, 1] = o4[2]
        out[s, 1, 1] = o4[3]
    return out
